# revision 65
# baseline (speedup 1.0000x reference)
"""BitAstroGPT forward pass on 8 TRN2 NeuronCores.

Sharding: data-parallel over batch (2 groups of 4 cores); within a group,
attention is head-sharded (4 heads per core, all 2048 queries) and the
residual/MLP are token-sharded (256 low-half + 256 high-half tokens per
core). Layer 0's normed activations are precomputed on the host (h0full),
so there are no prologue collectives. Per layer the schedule is:
attn-lo -> RS_a, attn-hi -> RS_b, MLP-lo -> AG_a(l+1), MLP-hi -> AG_b(l+1),
which keeps the single serial collective resource on the fixed order
RS_a, RS_b, AG_a', AG_b' and covers each AllGather's flight with the other
half's MLP matmuls. W1/W3 of the low half interleave into attention chunk
3's ACT-bound stretch; each rmsnorm's x^2/partition-sum pipeline is folded
into the producing W2/residual epilogues so only the ln/exp/broadcast tail
remains on the collective-trigger chain. The o-proj epilogue pre-scales by
gamma_v*gamma_o so the residual update is a plain add. Causal structure is
exact: query chunk c attends key tiles 0..4c+3 with multiplicative masks on
the 4 diagonal tiles; QK is emitted one tile ahead of PV; each pair's PV
accumulator and softmax-reciprocal broadcast share one PSUM bank. The LM
head streams vocab in 4-tile groups; rmsnorm(gf) is pipelined into the last
layer's W2 epilogues. rmsnorm uses ln+exp (rsqrt) to stay near the exp
activation table.

BitNet ternary quantization is exact in bf16; per-matrix gamma scales fold
into scalar immediates. Softmax runs without max-subtraction; denominators
come from a ones-column appended to V. Logits are emitted in bf16.
"""
import os
import numpy as np
import ml_dtypes

BF = ml_dtypes.bfloat16
V, B, T, D, L, H = 32000, 2, 2048, 1024, 4, 16
HD = 64
HID = 2730
HPAD = 2816           # 22 * 128
NMH = HPAD // 128     # 22
TC = 512              # local tokens per core (256 lo + 256 hi)
HC = 256
NET = D // 128        # 8 feature tiles
NKT = T // 128        # 16 key tiles (global)
NVT = V // 128        # 250 vocab tiles
EPS = 1e-6
GROUPS = [[0, 1, 2, 3], [4, 5, 6, 7]]

_cache = {}


def _quant(w):
    gamma = max(np.float32(np.mean(np.abs(w), dtype=np.float32)), np.float32(1e-5))
    tern = np.clip(np.round(np.float32(w) / gamma), -1.0, 1.0).astype(np.float32)
    return tern, float(gamma)


def _rope_tables():
    inv_freq = 1.0 / (10000.0 ** (np.arange(0, HD, 2, dtype=np.float32) / HD))
    t = np.arange(T, dtype=np.float32)
    freqs = np.einsum("i,j->ij", t, inv_freq)
    emb = np.concatenate([freqs, freqs], axis=-1)  # [T, 64]
    return np.cos(emb).astype(np.float32), np.sin(emb).astype(np.float32)


def _rot_lhs():
    # rot(q) = M @ q per 64-block; lhsT[e_in, e_out] = M[e_out, e_in]
    M = np.zeros((128, 128), np.float32)
    for blk in range(2):
        o = blk * 64
        for j in range(32):
            M[o + j, o + j + 32] = -1.0
            M[o + j + 32, o + j] = 1.0
    return np.ascontiguousarray(M.T).astype(BF)


def _build(scalars):
    import concourse.bacc as bacc
    import concourse.mybir as mybir
    import concourse.tile as tile

    F32 = mybir.dt.float32
    F32R = mybir.dt.float32r
    BF16 = mybir.dt.bfloat16
    AF = mybir.ActivationFunctionType
    OP = mybir.AluOpType
    es_l, vo_l, sil_l, m23_l = scalars

    nc = bacc.Bacc("TRN2", target_bir_lowering=False, debug=False, num_devices=8)

    xT0 = nc.dram_tensor("xT0", [D, TC], F32, kind="ExternalInput")
    h0full = nc.dram_tensor("h0full", [D, T], BF16, kind="ExternalInput")
    cosf = nc.dram_tensor("cosf", [128, T], BF16, kind="ExternalInput")
    sinf = nc.dram_tensor("sinf", [128, T], BF16, kind="ExternalInput")
    dmask = nc.dram_tensor("dmask", [128, 4 * 512], BF16, kind="ExternalInput")
    rlhs = nc.dram_tensor("rlhs", [128, 128], BF16, kind="ExternalInput")
    g1s = nc.dram_tensor("g1s", [128, L * NET], F32, kind="ExternalInput")
    g2s = nc.dram_tensor("g2s", [128, L * NET], F32, kind="ExternalInput")
    gfs = nc.dram_tensor("gfs", [128, NET], F32, kind="ExternalInput")
    wq = nc.dram_tensor("wq", [L, D, 256], BF16, kind="ExternalInput")
    wk = nc.dram_tensor("wk", [L, D, 256], BF16, kind="ExternalInput")
    wv = nc.dram_tensor("wv", [L, D, 256], BF16, kind="ExternalInput")
    wo = nc.dram_tensor("wo", [L, 256, D], BF16, kind="ExternalInput")
    w1t = nc.dram_tensor("w1t", [L, D, HPAD], BF16, kind="ExternalInput")
    w3t = nc.dram_tensor("w3t", [L, D, HPAD], BF16, kind="ExternalInput")
    w2t = nc.dram_tensor("w2t", [L, HPAD, D], BF16, kind="ExternalInput")
    wlm = nc.dram_tensor("wlm", [D, V], BF16, kind="ExternalInput")
    logitsT = nc.dram_tensor("logitsT", [V, TC], BF16, kind="ExternalOutput")

    with tile.TileContext(nc) as tc:
        with (
            tc.tile_pool(name="sb", bufs=3) as sb,
            tc.tile_pool(name="ps", bufs=3, space="PSUM") as ps,
            tc.tile_pool(name="dram", bufs=1, space="DRAM") as dram,
        ):
            # ---- prologue-critical loads first: layer-0 h columns, so
            # the first qkv matmuls start as soon as possible ----
            h_all = sb.tile([128, NET, T // 2], BF16, tag="hall", name="h_all",
                            bufs=1)
            for cb in range(2):
                nc.sync.dma_start(
                    h_all[:, :, cb * 512:(cb + 1) * 512],
                    h0full[:, cb * 512:(cb + 1) * 512]
                    .rearrange("(e p) t -> p e t", p=128))

            # ---- persistent constants ----
            ones_bf = sb.tile([128, 128], BF16, tag="ones", name="ones_bf", bufs=1)
            nc.vector.memset(ones_bf[:], 1.0)
            ones32 = sb.tile([128, 128], F32, tag="ones32", name="ones32", bufs=1)
            nc.vector.memset(ones32[:], 1.0)
            rlhs_sb = sb.tile([128, 128], BF16, tag="rlhs", name="rlhs_sb", bufs=1)
            nc.sync.dma_start(rlhs_sb[:], rlhs[:])
            cos_sb = sb.tile([128, T], BF16, tag="cos", name="cos_sb", bufs=1)
            nc.sync.dma_start(cos_sb[:], cosf[:])
            sin_sb = sb.tile([128, T], BF16, tag="sin", name="sin_sb", bufs=1)
            nc.sync.dma_start(sin_sb[:], sinf[:])
            mask_sb = sb.tile([128, 4, 512], BF16, tag="mask", name="mask_sb", bufs=1)
            nc.sync.dma_start(
                mask_sb[:], dmask[:].rearrange("p (d t) -> p d t", d=4))
            g1_sb = sb.tile([128, L * NET], F32, tag="g1", name="g1_sb", bufs=1)
            nc.sync.dma_start(g1_sb[:], g1s[:])
            g2_sb = sb.tile([128, L * NET], F32, tag="g2", name="g2_sb", bufs=1)
            nc.sync.dma_start(g2_sb[:], g2s[:])
            gf_sb = sb.tile([128, NET], F32, tag="gf", name="gf_sb", bufs=1)
            nc.sync.dma_start(gf_sb[:], gfs[:])

            eps_sb = sb.tile([1, 1], F32, tag="eps", name="eps_sb", bufs=1)
            nc.vector.memset(eps_sb[:], EPS)

            x_big = sb.tile([128, NET, TC], F32, tag="x", name="x_big", bufs=1)
            for i in range(NET):
                nc.sync.dma_start(x_big[:, i, :], xT0[i * 128:(i + 1) * 128, :])

            q_sb = sb.tile([128, 2, T], BF16, tag="qsb0", name="q_sb", bufs=1)
            k_sb = sb.tile([128, 2, T], BF16, tag="ksb", name="k_sb", bufs=1)
            # v token-major with ones column per local head (4 heads x 65)
            v_pad = sb.tile([128, NKT, 4 * 65], BF16, tag="vpad", name="v_pad", bufs=1)
            ones_view = v_pad[:].rearrange("p kt (h c) -> p kt h c", c=65)[:, :, :, 64:65]
            nc.vector.memset(ones_view, 1.0)
            y_sb = sb.tile([128, 2, T], BF16, tag="ysb", name="y_sb", bufs=1)
            hf = sb.tile([128, NET, TC], BF16, tag="hf", name="hf_big", bufs=1)

            # ---- helpers ----
            def proj(wslice, rhs, nk, n_m, epi, ncol, G, acc_tag):
                """out[m] = sum_k wslice(..)[:,m].T @ rhs(k); one weight DMA
                per (group, 8-ktile chunk)."""
                for g0 in range(0, n_m, G):
                    gm = min(G, n_m - g0)
                    accs = [ps.tile([128, ncol], F32, tag=acc_tag,
                                    name=f"acc{mi}", bufs=4)
                            for mi in range(gm)]
                    for kp in range(0, nk, 8):
                        kn = min(8, nk - kp)
                        w_sb = sb.tile([128, kn, gm * 128], BF16, tag="w",
                                       name="w_sb", bufs=4)
                        nc.sync.dma_start(
                            w_sb[:], wslice(kp, kn, g0, gm).rearrange(
                                "(k p) m -> p k m", p=128))
                        for ki in range(kn):
                            k = kp + ki
                            for mi in range(gm):
                                nc.tensor.matmul(
                                    accs[mi][:],
                                    w_sb[:, ki, mi * 128:(mi + 1) * 128],
                                    rhs(k), start=(k == 0),
                                    stop=(k == nk - 1))
                    for mi in range(gm):
                        epi(g0 + mi, accs[mi])

            def norm_x2(i, c0, ncol, ssum):
                """x2 of one feature tile + partition-sum into ssum (PSUM)."""
                x2 = sb.tile([128, ncol], BF16, tag="x2", name="x2", bufs=2)
                nc.any.tensor_mul(x2[:], x_big[:, i, c0:c0 + ncol],
                                  x_big[:, i, c0:c0 + ncol])
                nc.tensor.matmul(ssum[:], ones_bf[:, 0:1], x2[:],
                                 start=(i == 0), stop=(i == NET - 1))

            def norm_apply(ssum, g_base, g_off, c0, ncol, out_big, oc0=0):
                """inv = (ms+eps)^-1/2 via ln+exp (stays near the exp table),
                broadcast with an f32r matmul, apply with per-tile stt."""
                lnv = sb.tile([1, ncol], F32, tag="nrm", name="lnv", bufs=2)
                nc.scalar.activation(lnv[:], ssum[:], AF.Ln, bias=eps_sb[0:1, 0:1],
                                     scale=1.0 / D)
                inv = sb.tile([1, ncol], F32, tag="nrm", name="inv", bufs=2)
                nc.scalar.activation(inv[:], lnv[:], AF.Exp, scale=-0.5)
                rsig = ps.tile([128, ncol], F32, tag="acc", name="rsig", bufs=4)
                nc.tensor.matmul(rsig[:], ones32[0:1, :], inv[:],
                                 start=True, stop=True)
                for i in range(NET):
                    nc.vector.scalar_tensor_tensor(
                        out_big[:, i, oc0:oc0 + ncol], x_big[:, i, c0:c0 + ncol],
                        g_base[:, g_off + i:g_off + i + 1],
                        rsig[:], OP.mult, OP.mult)

            def rope_tile(src, cols, sink):
                rp = ps.tile([128, 512], F32, tag="acc", name="rotp", bufs=4)
                nc.tensor.matmul(rp[:], rlhs_sb[:], src[:], start=True, stop=True)
                t1 = sb.tile([128, 512], BF16, tag="rt", name="rt1", bufs=2)
                nc.any.tensor_mul(t1[:], src[:], cos_sb[:, cols])
                t2 = sb.tile([128, 512], BF16, tag="rt", name="rt2", bufs=2)
                nc.any.tensor_mul(t2[:], rp[:], sin_sb[:, cols])
                return sink(t1, t2)

            def unpack(agout, t0, rr=None):
                shards = range(4) if rr is None else (2 * rr, 2 * rr + 1)
                for r in shards:
                    nc.sync.dma_start(
                        h_all[:, :, r * 256:(r + 1) * 256],
                        agout[r * D:(r + 1) * D, :]
                        .rearrange("(e p) t -> p e t", p=128))

            def attn_pair(l, c, ft, hp):
                """attention for head 2*ft+hp, global query chunk c.

                One full PSUM bank per pair: PV accumulates into partitions
                0:65 (64 y rows + denominator), and the reciprocal broadcast
                lands in partitions 64:128 of the same bank, so each pair
                costs one bank and pairs double-buffer."""
                cols = slice(c * 512, (c + 1) * 512)
                nkt = 4 * (c + 1)
                h = 2 * ft + hp
                hsl = slice(hp * 64, (hp + 1) * 64)
                y_aug = ps.tile([128, 512], F32, tag="y", name="y_aug", bufs=2)
                p_tiles = []

                def qk(kt):
                    s_ps = ps.tile([128, 512], F32, tag="s", name="s_ps", bufs=2)
                    nc.tensor.matmul(
                        s_ps[:], k_sb[hsl, ft, kt * 128:(kt + 1) * 128],
                        q_sb[hsl, ft, cols], start=True, stop=True)
                    p_sb = sb.tile([128, 512], BF16, tag="p", name="p_sb", bufs=6)
                    nc.scalar.activation(p_sb[:], s_ps[:], AF.Exp, scale=es_l[l])
                    if kt >= 4 * c:
                        nc.any.tensor_mul(p_sb[:], p_sb[:],
                                             mask_sb[:, kt - 4 * c, :])
                    p_tiles.append(p_sb)

                def pv(kt):
                    nc.tensor.matmul(
                        y_aug[0:65, :], v_pad[:, kt, h * 65:(h + 1) * 65],
                        p_tiles[kt][:], start=(kt == 0), stop=(kt == nkt - 1))

                # emit QK one tile ahead of PV so the PE isn't stuck waiting
                # for exp(kt) right after QK(kt)
                qk(0)
                for kt in range(nkt):
                    if kt + 1 < nkt:
                        qk(kt + 1)
                    pv(kt)
                rec = sb.tile([1, 512], BF16, tag="rec", name="rec", bufs=2)
                with nc.allow_low_precision(reason="softmax denom reciprocal"):
                    nc.vector.reciprocal(rec[0:1, :], y_aug[64:65, :])
                nc.tensor.matmul(y_aug[64:128, :], ones_bf[0:1, 0:64],
                                 rec[0:1, :], start=True, stop=True)
                # DVE may read only one PSUM operand: stage the broadcast
                # through SBUF before the multiply.
                rh_sb = sb.tile([64, 512], F32, tag="rh", name="rh_sb", bufs=2)
                nc.any.tensor_copy(rh_sb[:], y_aug[64:128, :])
                nc.any.tensor_mul(
                    y_sb[hp * 64:(hp + 1) * 64, ft, cols],
                    y_aug[0:64, :], rh_sb[:])

            def o_chunk(l, c, rsin, wog):
                """o-proj partials for query chunk c -> rsin blocks; the
                vo scale folds into this epilogue (linear, pre-ReduceScatter)
                so the residual add is a plain tensor_add on any engine."""
                cols = slice(c * 512, (c + 1) * 512)
                osb = sb.tile([128, NET, 512], BF16, tag="osb", name="osb", bufs=1)
                for m in range(NET):
                    acc = ps.tile([128, 512], F32, tag="acc", name="oacc", bufs=4)
                    for k in range(2):
                        nc.tensor.matmul(acc[:], wog[:, k, m * 128:(m + 1) * 128],
                                         y_sb[:, k, cols],
                                         start=(k == 0), stop=(k == 1))
                    nc.any.tensor_scalar_mul(osb[:, m, :], acc[:], vo_l[l])
                j0 = 2 * (c % 2)
                for j in range(2):
                    nc.sync.dma_start(
                        rsin[(j0 + j) * D:(j0 + j + 1) * D, :]
                        .rearrange("(e p) t -> p e t", p=128),
                        osb[:, :, j * 256:(j + 1) * 256])

            def resid_add(rsout, c0, scale):
                rso = sb.tile([128, NET, HC], BF16, tag="rso", name="rso", bufs=2)
                nc.sync.dma_start(
                    rso[:], rsout[:].rearrange("(e p) t -> p e t", p=128))
                for i in range(NET):
                    nc.any.tensor_add(
                        x_big[:, i, c0:c0 + HC], rso[:, i, :],
                        x_big[:, i, c0:c0 + HC])

            def mlp_in_norm(l, c0, ssum_tag="y"):
                """post-resid rmsnorm(g2) -> hm.  ssum_tag picks the PSUM tag:
                "y" is only safe once the half's attention pairs are done;
                "acc" is safe between o_chunks (attention never touches acc)."""
                ssum = ps.tile([1, HC], F32, tag=ssum_tag, name="nssum",
                               bufs=2 if ssum_tag == "y" else 4)
                for i in range(NET):
                    norm_x2(i, c0, HC, ssum)
                hm = sb.tile([128, NET, HC], BF16, tag="hm", name="hm_big", bufs=2)
                norm_apply(ssum, g2_sb, l * NET, c0, HC, hm)
                return hm

            def mlp_w13_units(l, hm, prods):
                """W1/W3 + silu + product units (one per 4 m-tiles); appends
                prod tiles to `prods` as the units run."""
                for g0 in range(0, NMH, 4):
                    gm = min(4, NMH - g0)
                    yield lambda _g0=g0, _gm=gm: w13_group(l, hm, prods, _g0, _gm)

            def w13_group(l, hm, prods, g0, gm):
                if True:
                    s_tiles = []

                    def s_epi(m, acc, _l=l):
                        # silu(ga) = (ga/2)*(1+tanh(ga/2)); tanh lives in the
                        # same act table as exp, so no table reload.
                        # Copy acc out first so the PSUM bank frees
                        # immediately instead of through the whole chain.
                        a_sb = sb.tile([128, HC], BF16, tag="ab",
                                       name="a_sb", bufs=2)
                        nc.any.tensor_copy(a_sb[:], acc[:])
                        th = sb.tile([128, HC], BF16, tag="th", name="th",
                                     bufs=2)
                        nc.scalar.activation(th[:], a_sb[:], AF.Tanh,
                                             scale=sil_l[_l] * 0.5)
                        u = sb.tile([128, HC], BF16, tag="asb", name="asb",
                                    bufs=4)
                        nc.any.tensor_scalar_add(u[:], th[:], 1.0)
                        t = sb.tile([128, HC], BF16, tag="asb2", name="asb2",
                                    bufs=4)
                        nc.vector.scalar_tensor_tensor(
                            t[:], a_sb[:], sil_l[_l] * 0.5, u[:],
                            OP.mult, OP.mult)
                        s_tiles.append(t)
                    proj(lambda kp, kn, gg0, gm_, _l=l, _g=g0:
                         w1t[_l, kp * 128:(kp + kn) * 128,
                             _g * 128:(_g + gm_) * 128],
                         lambda k: hm[:, k, :], NET, gm, s_epi, HC, 4,
                         "acc")

                    def b_epi(m, acc, _s=s_tiles):
                        t = sb.tile([128, HC], BF16, tag="bsb", name="bsb",
                                    bufs=4)
                        nc.any.tensor_copy(t[:], acc[:])
                        pr = sb.tile([128, HC], BF16, tag="prod", name="prod",
                                     bufs=22)
                        nc.any.tensor_mul(pr[:], _s[m][:], t[:])
                        prods.append(pr)
                    proj(lambda kp, kn, gg0, gm_, _l=l, _g=g0:
                         w3t[_l, kp * 128:(kp + kn) * 128,
                             _g * 128:(_g + gm_) * 128],
                         lambda k: hm[:, k, :], NET, gm, b_epi, HC, 4, "acc")

            def mlp_w13(l, hm):
                prods = []
                for u in mlp_w13_units(l, hm, prods):
                    u()
                return prods

            def mlp_w2(l, c0, prods, agin=None, agout=None, hf_out=None):
                """W2 with the following norm pipelined into its epilogue:
                each x-tile update immediately feeds x2+ssum, so only the
                ln/exp/broadcast/apply tail remains after the last matmul.
                Sink: next layer's AG (agin/agout) or the final hf tile."""
                ssum = ps.tile([1, HC], F32, tag="y", name="w2ssum", bufs=2)

                def w2_epi(m, acc, _l=l, _c0=c0):
                    nc.vector.scalar_tensor_tensor(
                        x_big[:, m, _c0:_c0 + HC], acc[:], m23_l[_l],
                        x_big[:, m, _c0:_c0 + HC], OP.mult, OP.add)
                    norm_x2(m, _c0, HC, ssum)
                proj(lambda kp, kn, g0, gm, _l=l:
                     w2t[_l, kp * 128:(kp + kn) * 128,
                         g0 * 128:(g0 + gm) * 128],
                     lambda k: prods[k][:], NMH, NET, w2_epi, HC, 4, "acc")
                if agin is not None:
                    # the AllGather trigger chain is the layer critical path:
                    # keep the scheduler from queueing next-layer work ahead
                    # of it on the ACT/PE/DVE/SP queues.
                    with tc.high_priority():
                        hbig = sb.tile([128, NET, HC], BF16, tag="hn",
                                       name="hn_big", bufs=2)
                        norm_apply(ssum, g1_sb, (l + 1) * NET, c0, HC, hbig)
                        nc.sync.dma_start(
                            agin[:].rearrange("(e p) t -> p e t", p=128),
                            hbig[:])
                        nc.gpsimd.collective_compute(
                            "AllGather", mybir.AluOpType.bypass,
                            replica_groups=GROUPS, ins=[agin[:]], outs=[agout[:]])
                else:
                    norm_apply(ssum, gf_sb, 0, c0, HC, hf_out, oc0=c0)

            def qkv_chunk(l, c, wqg, wkg):
                cols = slice(c * 512, (c + 1) * 512)
                lcol = slice((c % 2) * 512, (c % 2) * 512 + 512)
                for wg, dst in ((wqg, q_sb), (wkg, k_sb)):
                    for m in range(2):
                        acc = ps.tile([128, 512], F32, tag="acc",
                                      name="qkacc", bufs=4)
                        for k in range(NET):
                            nc.tensor.matmul(
                                acc[:], wg[:, k, m * 128:(m + 1) * 128],
                                h_all[:, k, lcol],
                                start=(k == 0), stop=(k == NET - 1))
                        t = sb.tile([128, 512], BF16, tag="qks", name="qks",
                                    bufs=2)
                        nc.any.tensor_copy(t[:], acc[:])

                        def qksink(t1, t2, _m=m, _cols=cols, _dst=dst):
                            nc.any.tensor_add(_dst[:, _m, _cols],
                                                 t1[:], t2[:])
                        rope_tile(t, cols, qksink)

            def v_tiles(half, wvg, quarter):
                for tl in range(4 * quarter, 4 * quarter + 4):
                    tt = 8 * half + tl
                    vacc = ps.tile([128, 256], F32, tag="acc", name="vacc",
                                   bufs=4)
                    for k in range(NET):
                        nc.tensor.matmul(
                            vacc[:], h_all[:, k, tl * 128:(tl + 1) * 128],
                            wvg[:, k, :], start=(k == 0), stop=(k == NET - 1))
                    dst = v_pad[:].rearrange(
                        "p kt (h c) -> p kt h c", c=65)[:, tt, :, 0:64]
                    nc.any.tensor_copy(dst, vacc[:].rearrange(
                        "p (h c) -> p h c", c=64))

            def lm_group(g0, gm, halves):
                """LM-head vocab group: one weight DMA, matmuls over the
                requested column halves (0 = lo tokens, HC = hi tokens)."""
                w_sb = sb.tile([128, NET, gm * 128], BF16, tag="w",
                               name="w_sb", bufs=4)
                nc.sync.dma_start(
                    w_sb[:, :, 0:gm * 128],
                    wlm[:, g0 * 128:(g0 + gm) * 128]
                    .rearrange("(k p) m -> p k m", p=128))
                for c0, ncol in halves:
                    accs = [ps.tile([128, ncol], F32, tag="acc",
                                    name=f"lmacc{mi}", bufs=4)
                            for mi in range(gm)]
                    for k in range(NET):
                        for mi in range(gm):
                            nc.tensor.matmul(
                                accs[mi][:], w_sb[:, k, mi * 128:(mi + 1) * 128],
                                hf[:, k, c0:c0 + ncol],
                                start=(k == 0), stop=(k == NET - 1))
                    for mi in range(gm):
                        lg = sb.tile([128, ncol], BF16, tag="lg", name="lg",
                                     bufs=4)
                        nc.any.tensor_copy(lg[:], accs[mi][:])
                        nc.sync.dma_start(
                            logitsT[(g0 + mi) * 128:(g0 + mi + 1) * 128,
                                    c0:c0 + ncol], lg[:])

            # ---- prologue ----
            # Layer 0's normed activations come precomputed from the host
            # (h0full carries all T tokens), so there are no prologue
            # collectives: attention starts as soon as the first column
            # blocks of h0full land in SBUF.
            ag_bufs = [None]
            for l in range(1, L):
                ag_bufs.append((
                    dram.tile([D, HC], BF16, tag="aga", name=f"agin_a{l}"),
                    dram.tile([4 * D, HC], BF16, tag="agoa", name=f"agout_a{l}"),
                    dram.tile([D, HC], BF16, tag="agb", name=f"agin_b{l}"),
                    dram.tile([4 * D, HC], BF16, tag="agob", name=f"agout_b{l}"),
                ))

            def h0_load(half):
                base = half * (T // 2)
                for cb in range(2):
                    nc.sync.dma_start(
                        h_all[:, :, cb * 512:(cb + 1) * 512],
                        h0full[:, base + cb * 512:base + (cb + 1) * 512]
                        .rearrange("(e p) t -> p e t", p=128))

            # ---- layers ----
            # Per layer: attn-lo -> RS_a, attn-hi -> RS_b, then MLP-lo ->
            # AG_a(l+1) and MLP-hi -> AG_b(l+1); each AllGather's flight is
            # covered by the other half's MLP matmuls.  Collective order:
            # RS_a, RS_b, AG_a', AG_b'.
            def load_attn_weights(l):
                wqg = sb.tile([128, NET, 256], BF16, tag="wq", name="wqg", bufs=1)
                nc.sync.dma_start(wqg[:], wq[l].rearrange("(k p) m -> p k m", p=128))
                wkg = sb.tile([128, NET, 256], BF16, tag="wk", name="wkg", bufs=1)
                nc.sync.dma_start(wkg[:], wk[l].rearrange("(k p) m -> p k m", p=128))
                wvg = sb.tile([128, NET, 256], BF16, tag="wv", name="wvg", bufs=1)
                nc.sync.dma_start(wvg[:], wv[l].rearrange("(k p) m -> p k m", p=128))
                wog = sb.tile([128, 2, D], BF16, tag="wo", name="wog", bufs=1)
                nc.sync.dma_start(wog[:], wo[l].rearrange("(k p) m -> p k m", p=128))
                return wqg, wkg, wvg, wog

            for l in range(L):
                # scheduler-sim hint: keep layer l+1's instructions from being
                # queued ahead of layer l's collective-trigger tails
                tc.tile_set_cur_wait(float(l))
                rsin_a = dram.tile([4 * D, HC], BF16, tag="rsa", name="rsin_a")
                rsin_b = dram.tile([4 * D, HC], BF16, tag="rsb", name="rsin_b")
                rsout_a = dram.tile([D, HC], BF16, tag="rsoa", name="rsout_a")
                rsout_b = dram.tile([D, HC], BF16, tag="rsob", name="rsout_b")
                wqg, wkg, wvg, wog = load_attn_weights(l)

                def rs(rsin, rsout):
                    nc.gpsimd.collective_compute(
                        "ReduceScatter", mybir.AluOpType.add,
                        replica_groups=GROUPS, ins=[rsin[:]], outs=[rsout[:]])

                # lo-half attention, chunk-interleaved with qkv
                tc.tile_set_cur_wait(l + 0.0)
                if l != 0:
                    unpack(ag_bufs[l][1], 0, rr=0)
                qkv_chunk(l, 0, wqg, wkg)
                v_tiles(0, wvg, 0)
                if l != 0:
                    unpack(ag_bufs[l][1], 0, rr=1)
                for ft in range(2):
                    for hp in range(2):
                        attn_pair(l, 0, ft, hp)
                qkv_chunk(l, 1, wqg, wkg)
                v_tiles(0, wvg, 1)
                attn_pair(l, 1, 0, 0)
                o_chunk(l, 0, rsin_a, wog)
                for ft, hp in ((0, 1), (1, 0), (1, 1)):
                    attn_pair(l, 1, ft, hp)
                o_chunk(l, 1, rsin_a, wog)
                rs(rsin_a, rsout_a)

                # hi-half attention; lo resid slots in after chunk 2 so its
                # DVE work overlaps chunk 3's PE/ACT work.
                if l == 0:
                    h0_load(1)
                else:
                    unpack(ag_bufs[l][3], T // 2, rr=0)
                qkv_chunk(l, 2, wqg, wkg)
                v_tiles(1, wvg, 0)
                if l != 0:
                    unpack(ag_bufs[l][3], T // 2, rr=1)
                for ft in range(2):
                    for hp in range(2):
                        attn_pair(l, 2, ft, hp)
                qkv_chunk(l, 3, wqg, wkg)
                v_tiles(1, wvg, 1)
                o_chunk(l, 2, rsin_b, wog)
                # lo resid+norm now (ssum in acc tag: y is still cycling
                # through chunk 3's pairs), so W1/W3-lo groups can interleave
                # into chunk 3's ACT-bound stretch below.
                resid_add(rsout_a, 0, vo_l[l])
                hm_lo = mlp_in_norm(l, 0, ssum_tag="acc")
                prods_lo = []
                w13_lo = list(mlp_w13_units(l, hm_lo, prods_lo))
                for ft in range(2):
                    for hp in range(2):
                        attn_pair(l, 3, ft, hp)
                        if w13_lo:
                            w13_lo.pop(0)()
                o_chunk(l, 3, rsin_b, wog)
                rs(rsin_b, rsout_b)

                # rest of MLP-lo; hi resid+norm overlap on DVE; AG_a(l+1)
                # right after W2-lo, AG_b(l+1) right after W2-hi.
                for u in w13_lo:
                    u()
                resid_add(rsout_b, HC, vo_l[l])
                if l + 1 < L:
                    mlp_w2(l, 0, prods_lo, agin=ag_bufs[l + 1][0],
                           agout=ag_bufs[l + 1][1])
                else:
                    mlp_w2(l, 0, prods_lo, hf_out=hf)
                if l + 1 < L:
                    hm_hi = mlp_in_norm(l, HC)
                    prods_hi = mlp_w13(l, hm_hi)
                    mlp_w2(l, HC, prods_hi, agin=ag_bufs[l + 1][2],
                           agout=ag_bufs[l + 1][3])
                else:
                    # Last layer: the lo half of the LM head only needs
                    # hf-lo, so its vocab groups start during RS_b's flight
                    # and interleave with MLP-hi; the hi half follows, with
                    # the shared weight DMA reused where both halves run in
                    # the same group visit.
                    lm_plan = [(g0, min(4, NVT - g0))
                               for g0 in range(0, NVT, 4)]
                    gi = 0
                    hm_hi = mlp_in_norm(l, HC)
                    prods_hi = []
                    for u in mlp_w13_units(l, hm_hi, prods_hi):
                        u()

                    mlp_w2(l, HC, prods_hi, hf_out=hf)
                    n_lo_only = gi
                    for g0, gm in lm_plan[gi:]:
                        lm_group(g0, gm, ((0, TC),))
                    for g0, gm in lm_plan[:n_lo_only]:
                        lm_group(g0, gm, ((HC, HC),))

    nc.compile()
    return nc


def _prep(inputs):
    """Host-side prep: quantization, layouts, per-core in_maps."""
    idx = np.asarray(inputs["idx"])
    emb = np.asarray(inputs["emb"], np.float32)

    qw = {}
    gam = {}
    for name in ["Wq", "Wk", "Wv", "Wo", "W1", "W3", "W2"]:
        W = np.asarray(inputs[name], np.float32)
        qw[name] = []
        gam[name] = []
        for l in range(L):
            t, g = _quant(W[l])
            qw[name].append(t)
            gam[name].append(g)

    es_l = tuple(gam["Wq"][l] * gam["Wk"][l] / np.sqrt(HD) for l in range(L))
    vo_l = tuple(gam["Wv"][l] * gam["Wo"][l] for l in range(L))
    sil_l = tuple(gam["W1"][l] for l in range(L))
    m23_l = tuple(gam["W2"][l] * gam["W3"][l] for l in range(L))
    scalars = (es_l, vo_l, sil_l, m23_l)

    # shared weight arrays (transposed to lhsT layout [K, M])
    wq_a = np.stack([qw["Wq"][l].T for l in range(L)]).astype(BF)
    wk_a = np.stack([qw["Wk"][l].T for l in range(L)]).astype(BF)
    wv_a = np.stack([qw["Wv"][l].T for l in range(L)]).astype(BF)
    wo_a = np.stack([qw["Wo"][l].T for l in range(L)]).astype(BF)
    w1_a = np.zeros((L, D, HPAD), BF)
    w3_a = np.zeros((L, D, HPAD), BF)
    w2_a = np.zeros((L, HPAD, D), BF)
    for l in range(L):
        w1_a[l, :, :HID] = qw["W1"][l].T.astype(BF)
        w3_a[l, :, :HID] = qw["W3"][l].T.astype(BF)
        w2_a[l, :HID, :] = qw["W2"][l].T.astype(BF)
    wlm_a = np.ascontiguousarray(np.asarray(inputs["Wlm"], np.float32).T).astype(BF)

    def gcol(g):  # [L, D] -> [128, L*8]
        return np.ascontiguousarray(
            np.asarray(g, np.float32).reshape(-1, NET, 128).transpose(2, 0, 1)
            .reshape(128, -1))
    g1s_a = gcol(inputs["g1"])
    g2s_a = gcol(inputs["g2"])
    gfs_a = gcol(np.asarray(inputs["gf"], np.float32)[None])
    rlhs_a = _rot_lhs()

    cos, sin = _rope_tables()
    row = np.tile(np.arange(HD), 2)
    cos_fm = np.ascontiguousarray(cos[:, row].T).astype(BF)   # [128, T]
    sin_fm = np.ascontiguousarray(sin[:, row].T).astype(BF)

    # diagonal masks: mask[d][p, t] = (d*128 + p) <= t,  t in 0..511
    dm = np.zeros((128, 4, 512), np.float32)
    for d in range(4):
        dm[:, d, :] = (d * 128 + np.arange(128)[:, None]) <= np.arange(512)[None, :]
    dm_a = np.ascontiguousarray(dm.reshape(128, 4 * 512)).astype(BF)

    # host-side layer-0 norm: h0 = rmsnorm(emb[idx]) * g1[0], feature-major,
    # all T tokens (replaces the prologue AllGathers entirely)
    g1f = np.asarray(inputs["g1"], np.float32)
    h0f = []
    for b in range(B):
        xb = emb[idx[b]]                               # [T, D] f32
        inv = 1.0 / np.sqrt((xb * xb).mean(axis=1) + EPS)
        h0 = (xb * inv[:, None] * g1f[0][None, :]).T   # [D, T]
        h0f.append(np.ascontiguousarray(h0).astype(BF))

    in_maps = []
    for c in range(8):
        b, r = c // 4, c % 4
        toks = np.concatenate([
            idx[b, r * HC:(r + 1) * HC],
            idx[b, T // 2 + r * HC:T // 2 + (r + 1) * HC]])
        x0 = np.ascontiguousarray(emb[toks].T)  # [D, TC] f32
        in_maps.append({
            "xT0": x0, "h0full": h0f[b],
            "cosf": cos_fm, "sinf": sin_fm, "dmask": dm_a,
            "rlhs": rlhs_a, "g1s": g1s_a, "g2s": g2s_a, "gfs": gfs_a,
            "wq": np.ascontiguousarray(wq_a[:, :, r * 256:(r + 1) * 256]),
            "wk": np.ascontiguousarray(wk_a[:, :, r * 256:(r + 1) * 256]),
            "wv": np.ascontiguousarray(wv_a[:, :, r * 256:(r + 1) * 256]),
            "wo": np.ascontiguousarray(wo_a[:, r * 256:(r + 1) * 256, :]),
            "w1t": w1_a, "w3t": w3_a, "w2t": w2_a, "wlm": wlm_a,
        })
    return scalars, in_maps


def kernel(**inputs) -> np.ndarray:
    from concourse.bass_utils import run_bass_kernel_spmd

    scalars, in_maps = _prep(inputs)
    key = tuple(tuple(s) for s in scalars)
    if key not in _cache:
        _cache[key] = _build(scalars)
    nc = _cache[key]

    trace = bool(int(os.environ.get("KERNEL_TRACE", "0")))
    res = run_bass_kernel_spmd(nc, in_maps, core_ids=list(range(8)), trace=trace)
    kernel.last_result = res

    logits = np.empty((B, T, V), np.float32)
    for c in range(8):
        b, r = c // 4, c % 4
        out = np.asarray(res.results[c]["logitsT"], dtype=np.float32)  # [V, TC]
        logits[b, r * HC:(r + 1) * HC, :] = out[:, 0:HC].T
        logits[b, T // 2 + r * HC:T // 2 + (r + 1) * HC, :] = out[:, HC:TC].T
    return logits

